# revision 10
# baseline (speedup 1.0000x reference)
"""Grouped-experts SwiGLU MoE kernel for 8 Trainium2 NeuronCores.

Expert-parallel: core i owns expert i (E == n_cores == 8) and the 2048
tokens routed to it (tokens are pre-sorted by expert, even split).

Per-core math (Ti=2048, D=2048, H=5632):
    gate = x_i @ w1_i.T ; up = x_i @ w3_i.T
    h    = silu(gate) * up
    out  = h @ w2_i.T

Layout: TensorE contracts over the partition dim, so every operand is
staged contraction-major; all transposes/casts happen on the host so
every device DMA is a natural contiguous load:
    xT  (D, Ti)  bf16   GEMM1/3 moving operand
    w1T (D, H)   bf16   GEMM1 stationary operand
    w3T (D, H)   bf16   GEMM3 stationary operand
    w2T (H, D)   bf16   GEMM2 moving operand
Matmuls run in bf16 (full PE rate, and measurably lower PE power than
fp16 under all-8-core load -> higher sustained DVFS clock) with fp32
PSUM accumulation.  The SwiGLU intermediate h is produced transposed
(hT, [H, T] tiles, bf16 in SBUF) and consumed directly as the GEMM2
stationary operand, so no on-device transposes are needed anywhere.

Token blocks are TB=1024 (two per core), halving the weight re-stream
traffic vs TB=512 (162MB vs 277MB per core), which matters both for DMA
headroom and package power.  Pass A streams w1/w3 in HC=128 chunks,
computing gate/up for two 512-token sub-blocks per chunk (512-row
matmuls = one full PSUM bank, the per-instruction maximum).  Pass B
streams w2 in DB=256 column chunks, double-buffered so each chunk loads
during the whole previous chunk's compute.
"""

import numpy as np

import concourse.tile as tile
import concourse.mybir as mybir
from concourse import bacc
from concourse.bass_utils import run_bass_kernel_spmd

E, T, D, H = 8, 16384, 2048, 5632
NCORES = 8
TCORE = T // NCORES

_prog_cache: dict = {}


def _build_program(loop_n=1, TB=1024, HC=128, DB=256, TSUB=512,
                   interleave=False, split0=False, lowbits=True):
    """Build + schedule + compile the per-core Bass program (SPMD).

    loop_n > 1 wraps the whole body in a hardware loop that recomputes
    the identical result loop_n times — used only for timing runs.
    """
    import contextlib
    f32 = mybir.dt.float32
    dt = mybir.dt.bfloat16
    P = 128
    DO = D // P             # 16 contraction subtiles for GEMM1/3
    HO = H // P             # 44 contraction subtiles for GEMM2
    NTB = TCORE // TB       # 2 token blocks
    NHC = H // HC           # 44 w1/w3 chunks
    NHS = HC // P           # 1
    NDB = D // DB           # 8 d chunks
    NTS = TB // TSUB        # 2 moving sub-blocks in pass A
    NTT = TB // P           # 8 t tiles in pass B
    assert D % P == 0 and H % HC == 0 and HC % P == 0
    assert TCORE % TB == 0 and D % DB == 0 and TB % TSUB == 0

    nc = bacc.Bacc(None, target_bir_lowering=False)
    xT = nc.dram_tensor("xT", [D, TCORE], dt, kind="ExternalInput")
    w1T = nc.dram_tensor("w1T", [D, H], dt, kind="ExternalInput")
    w3T = nc.dram_tensor("w3T", [D, H], dt, kind="ExternalInput")
    w2T = nc.dram_tensor("w2T", [H, D], dt, kind="ExternalInput")
    odt = dt if lowbits else f32
    out = nc.dram_tensor("out", [TCORE, D], odt, kind="ExternalOutput")

    xTr = xT[:].rearrange("(do p) t -> p do t", p=P)
    w1Tr = w1T[:].rearrange("(do p) h -> p do h", p=P)
    w3Tr = w3T[:].rearrange("(do p) h -> p do h", p=P)
    w2Tr = w2T[:].rearrange("(ho p) d -> p ho d", p=P)
    silu = mybir.ActivationFunctionType.Silu

    with tile.TileContext(nc) as tc:
        with (
            tc.tile_pool(name="xpool", bufs=1) as xpool,
            tc.tile_pool(name="wpool", bufs=2) as wpool,
            tc.tile_pool(name="hpool", bufs=1) as hpool,
            tc.tile_pool(name="w2pool", bufs=2) as w2pool,
            tc.tile_pool(name="spool", bufs=3) as spool,
            tc.tile_pool(name="opool", bufs=3) as opool,
            tc.tile_pool(name="pgate", bufs=2, space="PSUM") as pgate,
            tc.tile_pool(name="pout", bufs=2, space="PSUM") as pout,
        ):
            loop_ctx = tc.For_i(0, loop_n, 1) if loop_n > 1 else \
                contextlib.nullcontext()
            with loop_ctx:
                for tb in range(NTB):
                    tsl_all = slice(tb * TB, (tb + 1) * TB)
                    x_t = xpool.tile([P, DO, TB], dt, tag="xt")

                    # interleave x pieces with the first weight chunk so
                    # the PE can start after ~1.5MB of DMA, not 5MB
                    def load_x(piece):
                        dsl = slice(piece * 4, (piece + 1) * 4)
                        nc.sync.dma_start(x_t[:, dsl, :],
                                          xTr[:, dsl, tsl_all])

                    load_x(0)
                    w1t0 = wpool.tile([P, DO, HC], dt, tag="w1t")
                    w3t0 = wpool.tile([P, DO, HC], dt, tag="w3t")
                    if split0:
                        # halve the first-chunk loads so the PE's first
                        # group starts a few us earlier on a cold start
                        h2 = HC // 2
                        nc.sync.dma_start(w1t0[:, :, 0:h2],
                                          w1Tr[:, :, 0:h2])
                        for piece in range(1, 4):
                            load_x(piece)
                        nc.sync.dma_start(w1t0[:, :, h2:HC],
                                          w1Tr[:, :, h2:HC])
                        nc.sync.dma_start(w3t0[:], w3Tr[:, :, 0:HC])
                    else:
                        nc.sync.dma_start(w1t0[:], w1Tr[:, :, 0:HC])
                        nc.sync.dma_start(w3t0[:], w3Tr[:, :, 0:HC])
                        for piece in range(1, 4):
                            load_x(piece)

                    hT = hpool.tile([P, HO, TB], dt, tag="ht")

                    # ---- pass A: hT[h, t] = silu(w1T.T x) * (w3T.T x) ----
                    for hc in range(NHC):
                        hsl_all = slice(hc * HC, (hc + 1) * HC)
                        if hc == 0:
                            w1t, w3t = w1t0, w3t0
                        else:
                            w1t = wpool.tile([P, DO, HC], dt, tag="w1t")
                            nc.sync.dma_start(w1t[:], w1Tr[:, :, hsl_all])
                            w3t = wpool.tile([P, DO, HC], dt, tag="w3t")
                            nc.sync.dma_start(w3t[:], w3Tr[:, :, hsl_all])
                        for hs in range(NHS):
                            hsl = slice(hs * P, (hs + 1) * P)
                            hrow = hc * NHS + hs
                            for ts in range(NTS):
                                tsl = slice(ts * TSUB, (ts + 1) * TSUB)
                                gate = pgate.tile([P, TSUB], f32, tag="gate")
                                up = pgate.tile([P, TSUB], f32, tag="up")
                                if interleave in (True, "A"):
                                    # gate/up pairs stream the same moving
                                    # x slice back-to-back: measurably
                                    # faster on HW (shared-operand reads)
                                    for do in range(DO):
                                        nc.tensor.matmul(
                                            gate[:], w1t[:, do, hsl],
                                            x_t[:, do, tsl],
                                            start=(do == 0),
                                            stop=(do == DO - 1))
                                        nc.tensor.matmul(
                                            up[:], w3t[:, do, hsl],
                                            x_t[:, do, tsl],
                                            start=(do == 0),
                                            stop=(do == DO - 1))
                                else:
                                    for do in range(DO):
                                        nc.tensor.matmul(
                                            gate[:], w1t[:, do, hsl],
                                            x_t[:, do, tsl],
                                            start=(do == 0),
                                            stop=(do == DO - 1))
                                    for do in range(DO):
                                        nc.tensor.matmul(
                                            up[:], w3t[:, do, hsl],
                                            x_t[:, do, tsl],
                                            start=(do == 0),
                                            stop=(do == DO - 1))
                                sil = spool.tile([P, TSUB],
                                                 dt if lowbits else f32,
                                                 tag="sil")
                                nc.scalar.activation(sil[:], gate[:], silu)
                                nc.vector.tensor_mul(
                                    hT[:, hrow, tsl], sil[:], up[:])

                    # ---- pass B: out[t, d] = hT.T @ w2T ----
                    for db in range(NDB):
                        dsl = slice(db * DB, (db + 1) * DB)
                        if DB == 512:
                            # single-buffered big chunk, loaded in 4 ho
                            # pieces so the next chunk's load overlaps the
                            # tail of this chunk's matmuls (subtile deps)
                            w2t = w2pool.tile([P, HO, DB], dt, tag="w2t",
                                              bufs=1)
                            for piece in range(4):
                                psl = slice(piece * (HO // 4),
                                            (piece + 1) * (HO // 4))
                                nc.sync.dma_start(w2t[:, psl, :],
                                                  w2Tr[:, psl, dsl])
                        else:
                            w2t = w2pool.tile([P, HO, DB], dt, tag="w2t")
                            nc.sync.dma_start(w2t[:], w2Tr[:, :, dsl])
                        if interleave in (True, "B"):
                            # two adjacent token-tile groups share each
                            # moving w2 slice back-to-back
                            for tp in range(NTT // 2):
                                tsls = [slice((2 * tp + j) * P,
                                              (2 * tp + j + 1) * P)
                                        for j in range(2)]
                                # full PSUM bank each so groups never
                                # share accumulation state in one bank
                                opss = [pout.tile([P, 512], f32,
                                                  tag=f"ops{j}",
                                                  name=f"ops{j}_{tb}_{db}_{tp}")
                                        for j in range(2)]
                                for ho in range(HO):
                                    for j in range(2):
                                        nc.tensor.matmul(
                                            opss[j][:, 0:DB],
                                            hT[:, ho, tsls[j]],
                                            w2t[:, ho, :],
                                            start=(ho == 0),
                                            stop=(ho == HO - 1))
                                for j in range(2):
                                    tt = 2 * tp + j
                                    osl = slice(tb * TB + tt * P,
                                                tb * TB + (tt + 1) * P)
                                    ob = opool.tile([P, DB], odt, tag="ob")
                                    nc.vector.tensor_copy(
                                        ob[:], opss[j][:, 0:DB])
                                    nc.sync.dma_start(out[osl, dsl], ob[:])
                        else:
                            for tt in range(NTT):
                                tsl = slice(tt * P, (tt + 1) * P)
                                # padded to a full PSUM bank so two groups
                                # never share accumulation state in a bank
                                ops = pout.tile([P, 512], f32, tag="ops")
                                for ho in range(HO):
                                    nc.tensor.matmul(
                                        ops[:, 0:DB], hT[:, ho, tsl],
                                        w2t[:, ho, :],
                                        start=(ho == 0), stop=(ho == HO - 1))
                                osl = slice(tb * TB + tt * P,
                                            tb * TB + (tt + 1) * P)
                                ob = opool.tile([P, DB], odt, tag="ob")
                                nc.vector.tensor_copy(ob[:], ops[:, 0:DB])
                                nc.sync.dma_start(out[osl, dsl], ob[:])
    nc.compile()
    return nc


def _get_program(**kw):
    key = tuple(sorted(kw.items()))
    if key not in _prog_cache:
        _prog_cache[key] = _build_program(**kw)
    return _prog_cache[key]


def _host_prep(x_i, w1_i, w2_i, w3_i):
    import ml_dtypes
    bf16 = ml_dtypes.bfloat16
    return {
        "xT": np.ascontiguousarray(x_i.T).astype(bf16),
        "w1T": np.ascontiguousarray(w1_i.T).astype(bf16),
        "w3T": np.ascontiguousarray(w3_i.T).astype(bf16),
        "w2T": np.ascontiguousarray(w2_i.T).astype(bf16),
    }


def _numpy_fallback(x, w1, w2, w3, counts):
    outs = []
    start = 0
    for e in range(len(counts)):
        n = int(counts[e])
        xe = x[start:start + n]
        gate = xe @ w1[e].T
        up = xe @ w3[e].T
        h = (gate / (1.0 + np.exp(-gate))) * up
        outs.append(h @ w2[e].T)
        start += n
    return np.concatenate(outs, axis=0).astype(np.float32)


def kernel(x, w1, w2, w3, num_tokens_per_expert, _trace=False):
    x = np.asarray(x, dtype=np.float32)
    w1 = np.asarray(w1, dtype=np.float32)
    w2 = np.asarray(w2, dtype=np.float32)
    w3 = np.asarray(w3, dtype=np.float32)
    counts = np.asarray(num_tokens_per_expert).astype(np.int64)

    if not (len(counts) == E and np.all(counts == TCORE)):
        return _numpy_fallback(x, w1, w2, w3, counts)

    nc = _get_program()
    in_maps = [
        _host_prep(x[i * TCORE:(i + 1) * TCORE], w1[i], w2[i], w3[i])
        for i in range(NCORES)
    ]
    res = run_bass_kernel_spmd(
        nc, in_maps, core_ids=list(range(NCORES)), trace=_trace
    )
    out = np.concatenate([r["out"] for r in res.results],
                         axis=0).astype(np.float32)
    if _trace:
        return out, res
    return out
